# revision 13
# baseline (speedup 1.0000x reference)
"""AdaptedAttention (llama-adapter) Trainium2 kernel, 8-core token-data-parallel.

Strategy:
  - 8192 tokens (B*S) split 1024/core; q_w/o_w replicated, k_w/v_w head-sharded.
  - All activations kept transposed [feature, token] so weights are the
    stationary matmul operand (lhsT) and tokens stream as the moving operand.
  - float32r (TF32-like, full-rate) matmuls; scales/gate/1/sqrt(hd) folded
    into weights on host; biases applied via K=1 matmuls.
  - Adapter K/V computed per-core for its 4 heads, AllGathered (tiny).
  - Softmax denominator broadcast via ones-matrix matmul; ctx bounced through
    DRAM so x and ctx are never SBUF-resident together.
"""
import os
import numpy as np

import concourse.bass as bass
import concourse.tile as tile
from concourse import mybir
from concourse.bass_utils import run_bass_kernel_spmd

F32 = mybir.dt.float32
F32R = mybir.dt.float32r
P = 128

NUM_HEADS = 32
B, S, H, L = 4, 2048, 4096, 64
HD = H // NUM_HEADS            # 128
NC = 8
TOK = (B * S) // NC            # 1024 tokens per core
KT = H // P                    # 32 contraction tiles
MT = H // P                    # 32 output feature tiles
NB = TOK // 512                # 2 moving-operand chunks of 512
HPC = NUM_HEADS // NC          # 4 heads per core
DSL = HPC * HD                 # 512 adapter dims per core

_PATCHED = False


def _patch_tile():
    """TRN2 instructions have one hw wait slot; walrus rejects multi-wait
    matmuls and the kernel-tail drain. Hoist extra waits onto NoOps."""
    global _PATCHED
    if _PATCHED:
        return
    _PATCHED = True
    import concourse.tile as _tile
    from concourse.vector_clock import ScopedClock

    _orig_commit = _tile.TileContext._commit_instruction

    def _patched_commit(self, inst, lazy_reg_writes=True):
        si = getattr(inst, "sync_info", None)
        if (
            si is not None
            and si.on_wait
            and len(si.on_wait) > 1
            and inst.engine != mybir.EngineType.Unassigned
            and not isinstance(inst, mybir.InstNoOp)
        ):
            waits = list(si.on_wait)
            for w in waits[:-1]:
                nop = mybir.InstNoOp(
                    name=self.nc.get_next_instruction_name(),
                    ins=[], outs=[], bass_nofuse=True,
                )
                nop.engine = inst.engine
                nop.sync_info = mybir.SyncInfo(on_wait=[w], on_update=[])
                _orig_commit(self, nop, lazy_reg_writes=False)
            inst.sync_info = mybir.SyncInfo(
                on_wait=waits[-1:], on_update=list(si.on_update or [])
            )
        return _orig_commit(self, inst, lazy_reg_writes=lazy_reg_writes)

    def _patched_drain_and_barrier(self, tick_clock, wait_clock):
        nc = self.nc
        carrier = nc.sync.nop(nofuse=True)
        wait_clock.add_sem_waits(
            carrier.ins, ScopedClock({None: tick_clock.global_clock})
        )
        si = carrier.ins.sync_info
        waits = list(si.on_wait) if si and si.on_wait else []
        if len(waits) > 1:
            carrier.ins.sync_info = mybir.SyncInfo(
                on_wait=waits[:1], on_update=list(si.on_update or [])
            )
            for w in waits[1:]:
                extra = nc.sync.nop(nofuse=True)
                extra.ins.sync_info = mybir.SyncInfo(on_wait=[w], on_update=[])
        nc.sync.drain()
        nc.all_engine_barrier()
        assert self.sems is not None
        popped = nc._tile_sem_poison_stack.pop()
        assert popped is self._sem_poison
        nc.clear_and_free_semaphores(list(self.sems.allocated().values()))
        nc.all_engine_barrier()

    _tile.TileContext._commit_instruction = _patched_commit
    _tile.TileContext._drain_and_barrier = _patched_drain_and_barrier


def build_nc():
    _patch_tile()
    from contextlib import ExitStack

    nc = bass.Bass(target_bir_lowering=False)

    xT = nc.declare_dram_parameter("xT", [H, TOK], F32R, isOutput=False)
    qwB = nc.declare_dram_parameter("qwB", [MT, 8, P, 4, P], F32R, isOutput=False)
    owB = nc.declare_dram_parameter("owB", [MT, 8, P, 4, P], F32R, isOutput=False)
    kwT = nc.declare_dram_parameter("kwT", [H, DSL], F32R, isOutput=False)
    vwT = nc.declare_dram_parameter("vwT", [H, DSL], F32R, isOutput=False)
    prT = nc.declare_dram_parameter("prT", [H, L], F32R, isOutput=False)
    cosT = nc.declare_dram_parameter("cosT", [HD, TOK], F32, isOutput=False)
    srotT = nc.declare_dram_parameter("srotT", [HD, TOK], F32, isOutput=False)
    baseT = nc.declare_dram_parameter("baseT", [H, TOK], F32, isOutput=False)
    qb = nc.declare_dram_parameter("qb", [1, H], F32R, isOutput=False)
    ob = nc.declare_dram_parameter("ob", [1, H], F32R, isOutput=False)
    kb = nc.declare_dram_parameter("kb", [1, DSL], F32R, isOutput=False)
    vb = nc.declare_dram_parameter("vb", [1, DSL], F32R, isOutput=False)
    outT = nc.declare_dram_parameter("outT", [H, TOK], F32, isOutput=True)

    with tile.TileContext(nc) as tc:
        es = ExitStack()
        persist = es.enter_context(tc.tile_pool(name="persist", bufs=1))
        dram = es.enter_context(tc.tile_pool(name="dram", bufs=1, space="DRAM"))
        k_bnc = dram.tile([DSL, L], F32R, name="k_bnc")
        v_bnc = dram.tile([L, DSL], F32R, name="v_bnc")
        k_gat = dram.tile([H, L], F32R, name="k_gat", addr_space="Shared")
        v_gat = dram.tile([NC * L, DSL], F32R, name="v_gat", addr_space="Shared")
        ctxd = dram.tile([H, TOK], F32R, name="ctxd")

        # ---- persistent tiles ----
        cos_sb = persist.tile([HD, TOK], F32, name="cos_sb")
        nc.sync.dma_start(out=cos_sb[:], in_=cosT[:])
        srot_sb = persist.tile([HD, TOK], F32, name="srot_sb")
        nc.sync.dma_start(out=srot_sb[:], in_=srotT[:])
        katt = persist.tile([P, NUM_HEADS * L], F32R, name="katt")
        ones1r = persist.tile([1, TOK], F32R, name="ones1r")
        onesm = persist.tile([L, P], F32R, name="onesm")

        # ---- adapter K/V for this core's heads (transient pools) ----
        ad_es = ExitStack()
        apsum = ad_es.enter_context(tc.tile_pool(name="apsum", bufs=5, space="PSUM"))
        apool = ad_es.enter_context(tc.tile_pool(name="adpool", bufs=1))
        kwpool = ad_es.enter_context(tc.tile_pool(name="kwpool", bufs=3))

        ones_f = apool.tile([1, TOK], F32, name="ones_f")
        nc.vector.memset(ones_f[:], 1.0)
        nc.vector.tensor_copy(out=ones1r[:], in_=ones_f[:])
        onesm_f = apool.tile([L, P], F32, name="onesm_f")
        nc.vector.memset(onesm_f[:], 1.0)
        nc.vector.tensor_copy(out=onesm[:], in_=onesm_f[:])

        kb_sb = apool.tile([1, DSL], F32R, name="kb_sb")
        nc.sync.dma_start(out=kb_sb[:], in_=kb[:])
        vb_sb = apool.tile([1, DSL], F32R, name="vb_sb")
        nc.sync.dma_start(out=vb_sb[:], in_=vb[:])
        pr_sb = apool.tile([P, KT * L], F32R, name="pr_sb")
        for k in range(KT):
            nc.sync.dma_start(out=pr_sb[:, k * L:(k + 1) * L],
                              in_=prT[k * P:(k + 1) * P, :])

        # kT slice [DSL, L] ([d, l] layout) and v slice [L, DSL] ([l, d])
        pks = [apsum.tile([P, 512], F32, name=f"pk_{md}", tag="aps")
               for md in range(HPC)]
        pv = apsum.tile([P, 512], F32, name="pv", tag="aps")
        for k in range(KT):
            kwt = kwpool.tile([P, DSL], F32R, name=f"kwt_{k}", tag="kw")
            nc.gpsimd.dma_start(out=kwt[:], in_=kwT[k * P:(k + 1) * P, :])
            for md in range(HPC):
                nc.tensor.matmul(
                    out=pks[md][:, :L],
                    lhsT=kwt[:, md * P:(md + 1) * P],
                    rhs=pr_sb[:, k * L:(k + 1) * L],
                    start=(k == 0), stop=False,
                )
            vwt = kwpool.tile([P, DSL], F32R, name=f"vwt_{k}", tag="kw")
            nc.gpsimd.dma_start(out=vwt[:], in_=vwT[k * P:(k + 1) * P, :])
            nc.tensor.matmul(
                out=pv[:L, :DSL],
                lhsT=pr_sb[:, k * L:(k + 1) * L],
                rhs=vwt[:],
                start=(k == 0), stop=False,
            )
        for md in range(HPC):
            nc.tensor.matmul(
                out=pks[md][:, :L],
                lhsT=kb_sb[:, md * P:(md + 1) * P],
                rhs=ones1r[:, :L],
                start=False, stop=True,
            )
        nc.tensor.matmul(
            out=pv[:L, :DSL],
            lhsT=ones1r[:, :L],
            rhs=vb_sb[:],
            start=False, stop=True,
        )
        ksl_sb = apool.tile([P, HPC * L], F32R, name="ksl_sb")
        for md in range(HPC):
            nc.scalar.copy(out=ksl_sb[:, md * L:(md + 1) * L], in_=pks[md][:, :L])
            nc.sync.dma_start(out=k_bnc[md * P:(md + 1) * P, :],
                              in_=ksl_sb[:, md * L:(md + 1) * L])
        vsl_sb = apool.tile([L, DSL], F32R, name="vsl_sb")
        nc.scalar.copy(out=vsl_sb[:], in_=pv[:L, :DSL])
        nc.sync.dma_start(out=v_bnc[:], in_=vsl_sb[:])

        # all-gather adapter K/V across the 8 cores
        nc.gpsimd.collective_compute(
            "AllGather", mybir.AluOpType.bypass,
            replica_groups=[list(range(NC))],
            ins=[k_bnc[:]], outs=[k_gat[:]],
        )
        nc.gpsimd.collective_compute(
            "AllGather", mybir.AluOpType.bypass,
            replica_groups=[list(range(NC))],
            ins=[v_bnc[:]], outs=[v_gat[:]],
        )
        for h in range(NUM_HEADS):
            nc.gpsimd.dma_start(out=katt[:, h * L:(h + 1) * L],
                                in_=k_gat[h * P:(h + 1) * P, :])

        ad_es.close()

        # ---- phase 1: q-proj + RoPE + per-head adapter attention ----
        ph_es = ExitStack()
        psA = ph_es.enter_context(tc.tile_pool(name="psA", bufs=2, space="PSUM"))
        psB = ph_es.enter_context(tc.tile_pool(name="psB", bufs=2, space="PSUM"))
        wpool = ph_es.enter_context(tc.tile_pool(name="wpool", bufs=8))
        bias_p = ph_es.enter_context(tc.tile_pool(name="bias_p", bufs=2))
        xt_es = ExitStack()
        xt_pool = xt_es.enter_context(tc.tile_pool(name="xt", bufs=1))
        rpool = xt_es.enter_context(tc.tile_pool(name="rpool", bufs=1))
        tpool = xt_es.enter_context(tc.tile_pool(name="tpool", bufs=1))
        qw_pref = {}
        for h in range(1):
            for kg in range(8):
                wb = wpool.tile([P, 4, P], F32R, name=f"qw_{h}_{kg}", tag="qw")
                eng = (nc.sync, nc.scalar, nc.gpsimd)[kg % 3]
                eng.dma_start(out=wb[:], in_=qwB[h, kg])
                qw_pref[(h, kg)] = wb
        xt_tiles = []
        for k in range(KT):
            t = xt_pool.tile([P, TOK], F32R, name=f"xt_{k}", tag=f"xt_{k}")
            eng = (nc.sync, nc.scalar, nc.gpsimd)[k % 3]
            eng.dma_start(out=t[:], in_=xT[k * P:(k + 1) * P, :])
            xt_tiles.append(t)

        for h in range(MT):  # m-tile h == head h
            pq = psA.tile([P, TOK], F32, name=f"pq_{h}", tag="psA")
            for kg in range(8):
                if (h, kg) in qw_pref:
                    wb = qw_pref[(h, kg)]
                else:
                    wb = wpool.tile([P, 4, P], F32R, name=f"qw_{h}_{kg}", tag="qw")
                    eng = (nc.sync, nc.scalar, nc.gpsimd)[kg % 3]
                    eng.dma_start(out=wb[:], in_=qwB[h, kg])
                for kk in range(4):
                    k = kg * 4 + kk
                    for j in range(NB):
                        nc.tensor.matmul(
                            out=pq[:, j * 512:(j + 1) * 512],
                            lhsT=wb[:, kk, :],
                            rhs=xt_tiles[k][:, j * 512:(j + 1) * 512],
                            start=(k == 0), stop=False,
                        )
            qbt = bias_p.tile([1, P], F32R, name=f"qbt_{h}", tag="qbt")
            nc.sync.dma_start(out=qbt[:], in_=qb[0:1, h * P:(h + 1) * P])
            for j in range(NB):
                nc.tensor.matmul(
                    out=pq[:, j * 512:(j + 1) * 512],
                    lhsT=qbt[:],
                    rhs=ones1r[:, j * 512:(j + 1) * 512],
                    start=False, stop=True,
                )
            q_sb = rpool.tile([P, TOK], F32, name=f"q_sb_{h}", tag="q_sb")
            nc.scalar.copy(out=q_sb[:], in_=pq[:])

            # RoPE: qr = q*cos + rotate_half(q)*srot
            rot = rpool.tile([P, TOK], F32, name=f"rot_{h}", tag="rot")
            nc.scalar.dma_start(out=rot[0:64, :], in_=q_sb[64:128, :])
            nc.scalar.dma_start(out=rot[64:128, :], in_=q_sb[0:64, :])
            t2 = rpool.tile([P, TOK], F32, name=f"t2_{h}", tag="t2")
            nc.vector.tensor_mul(out=t2[:], in0=q_sb[:], in1=cos_sb[:])
            t1 = rpool.tile([P, TOK], F32, name=f"t1_{h}", tag="t1")
            nc.vector.tensor_mul(out=t1[:], in0=rot[:], in1=srot_sb[:])
            qr = rpool.tile([P, TOK], F32R, name=f"qr_{h}", tag="qr", bufs=1)
            nc.vector.tensor_add(out=qr[:], in0=t2[:], in1=t1[:])

            # attention vs adapter prompt (L=64); no max-subtraction needed
            sc = psB.tile([P, TOK], F32, name=f"sc_{h}", tag="psB")
            for j in range(NB):
                nc.tensor.matmul(
                    out=sc[:L, j * 512:(j + 1) * 512],
                    lhsT=katt[:, h * L:(h + 1) * L],
                    rhs=qr[:, j * 512:(j + 1) * 512],
                    start=True, stop=True,
                )
            expt = tpool.tile([L, TOK], F32R, name=f"expt_{h}", tag="expt",
                              bufs=2)
            nc.scalar.activation(expt[:], sc[:L, :],
                                 mybir.ActivationFunctionType.Exp)
            den = psB.tile([P, TOK], F32, name=f"den_{h}", tag="psB")
            for j in range(NB):
                nc.tensor.matmul(
                    out=den[:, j * 512:(j + 1) * 512],
                    lhsT=onesm[:],
                    rhs=expt[:, j * 512:(j + 1) * 512],
                    start=True, stop=True,
                )
            recip = tpool.tile([P, TOK], F32, name=f"recip_{h}", tag="recip")
            nc.vector.reciprocal(out=recip[:], in_=den[:])
            c_, hh_ = divmod(h, HPC)
            vh = tpool.tile([L, HD], F32R, name=f"vh_{h}", tag="vh", bufs=3)
            nc.gpsimd.dma_start(
                out=vh[:],
                in_=v_gat[c_ * L:(c_ + 1) * L, hh_ * HD:(hh_ + 1) * HD],
            )
            ctxp = psB.tile([P, TOK], F32, name=f"ctxp_{h}", tag="psB")
            for j in range(NB):
                nc.tensor.matmul(
                    out=ctxp[:, j * 512:(j + 1) * 512],
                    lhsT=vh[:],
                    rhs=expt[:, j * 512:(j + 1) * 512],
                    start=True, stop=True,
                )
            ctx_sb = tpool.tile([P, TOK], F32R, name=f"ctx_sb_{h}", tag="ctx_sb")
            nc.vector.tensor_mul(out=ctx_sb[:], in0=ctxp[:], in1=recip[:])
            (nc.sync, nc.scalar)[h % 2].dma_start(out=ctxd[h * P:(h + 1) * P, :], in_=ctx_sb[:])

        xt_es.close()

        # ---- phase 2: o-proj + bias + base ----
        p2_es = ExitStack()
        ctx_pool = p2_es.enter_context(tc.tile_pool(name="ctxr", bufs=1))
        bpool = p2_es.enter_context(tc.tile_pool(name="bpool", bufs=3))

        ow_pref = {}
        for m in range(1):
            for kg in range(8):
                wb = wpool.tile([P, 4, P], F32R, name=f"ow_{m}_{kg}", tag="qw")
                eng = (nc.sync, nc.scalar, nc.gpsimd)[kg % 3]
                eng.dma_start(out=wb[:], in_=owB[m, kg])
                ow_pref[(m, kg)] = wb
        ctx_tiles = []
        for k in range(KT):
            t = ctx_pool.tile([P, TOK], F32R, name=f"ctxr_{k}", tag=f"ctxr_{k}")
            eng = (nc.sync, nc.scalar, nc.gpsimd)[k % 3]
            eng.dma_start(out=t[:], in_=ctxd[k * P:(k + 1) * P, :])
            ctx_tiles.append(t)

        for m in range(MT):
            po = psA.tile([P, TOK], F32, name=f"po_{m}", tag="psA")
            for kg in range(8):
                if (m, kg) in ow_pref:
                    wb = ow_pref[(m, kg)]
                else:
                    wb = wpool.tile([P, 4, P], F32R, name=f"ow_{m}_{kg}", tag="qw")
                    eng = (nc.sync, nc.scalar, nc.gpsimd)[kg % 3]
                    eng.dma_start(out=wb[:], in_=owB[m, kg])
                for kk in range(4):
                    k = kg * 4 + kk
                    for j in range(NB):
                        nc.tensor.matmul(
                            out=po[:, j * 512:(j + 1) * 512],
                            lhsT=wb[:, kk, :],
                            rhs=ctx_tiles[k][:, j * 512:(j + 1) * 512],
                            start=(k == 0), stop=False,
                        )
            obt = bias_p.tile([1, P], F32R, name=f"obt_{m}", tag="qbt")
            nc.sync.dma_start(out=obt[:], in_=ob[0:1, m * P:(m + 1) * P])
            for j in range(NB):
                nc.tensor.matmul(
                    out=po[:, j * 512:(j + 1) * 512],
                    lhsT=obt[:],
                    rhs=ones1r[:, j * 512:(j + 1) * 512],
                    start=False, stop=True,
                )
            bt = bpool.tile([P, TOK], F32, name=f"bt_{m}", tag="bt")
            nc.scalar.dma_start(out=bt[:], in_=baseT[m * P:(m + 1) * P, :])
            os_ = bpool.tile([P, TOK], F32, name=f"os_{m}", tag="os")
            nc.vector.tensor_add(out=os_[:], in0=po[:], in1=bt[:])
            nc.gpsimd.dma_start(out=outT[m * P:(m + 1) * P, :], in_=os_[:])

        p2_es.close()
        ph_es.close()
        es.close()
    return nc


_NC_CACHE = None


def kernel(hidden_states, position_ids, base_output, cos, sin,
           q_w, k_w, v_w, o_w,
           q_scale, k_scale, v_scale, o_scale,
           q_bias, k_bias, v_bias, o_bias,
           adaption_prompt, adaption_gate):
    global _NC_CACHE

    hidden_states = np.asarray(hidden_states, dtype=np.float32)
    base_output = np.asarray(base_output, dtype=np.float32)
    pos = np.asarray(position_ids).reshape(-1).astype(np.int64)
    cos = np.asarray(cos, dtype=np.float32)
    sin = np.asarray(sin, dtype=np.float32)
    q_w = np.asarray(q_w, dtype=np.float32)
    k_w = np.asarray(k_w, dtype=np.float32)
    v_w = np.asarray(v_w, dtype=np.float32)
    o_w = np.asarray(o_w, dtype=np.float32)

    X = hidden_states.reshape(B * S, H)
    BASE = base_output.reshape(B * S, H)
    cosg = cos[0, 0][pos]                     # [B*S, HD]
    sing = sin[0, 0][pos]
    sr = sing.copy()
    sr[:, :HD // 2] *= -1.0                   # sign for rotate_half product

    inv = 1.0 / np.sqrt(HD)
    gate = float(np.asarray(adaption_gate).reshape(-1)[0])
    q_wT = np.ascontiguousarray((q_w * np.asarray(q_scale)[None, :]).T)
    o_wT = np.ascontiguousarray((o_w * np.asarray(o_scale)[None, :]).T)
    k_wT = np.ascontiguousarray((k_w * np.asarray(k_scale)[None, :]).T * inv)
    v_wT = np.ascontiguousarray((v_w * np.asarray(v_scale)[None, :]).T * gate)
    kbf = (np.asarray(k_bias) * inv).astype(np.float32).reshape(1, H)
    vbf = (np.asarray(v_bias) * gate).astype(np.float32).reshape(1, H)
    prT = np.ascontiguousarray(np.asarray(adaption_prompt, dtype=np.float32)[0].T)
    # blocked stationary layouts: [m, kg, 512, 128]
    qwB_ = np.ascontiguousarray(
        q_wT.reshape(8, 4, P, MT, P).transpose(3, 0, 2, 1, 4))
    owB_ = np.ascontiguousarray(
        o_wT.reshape(8, 4, P, MT, P).transpose(3, 0, 2, 1, 4))

    in_maps = []
    for c in range(NC):
        sl = slice(c * TOK, (c + 1) * TOK)
        dsl = slice(c * DSL, (c + 1) * DSL)
        in_maps.append({
            "xT": np.ascontiguousarray(X[sl].T),
            "baseT": np.ascontiguousarray(BASE[sl].T),
            "cosT": np.ascontiguousarray(cosg[sl].T),
            "srotT": np.ascontiguousarray(sr[sl].T),
            "qwB": qwB_,
            "owB": owB_,
            "kwT": np.ascontiguousarray(k_wT[:, dsl]),
            "vwT": np.ascontiguousarray(v_wT[:, dsl]),
            "prT": prT,
            "qb": np.asarray(q_bias, dtype=np.float32).reshape(1, H),
            "ob": np.asarray(o_bias, dtype=np.float32).reshape(1, H),
            "kb": np.ascontiguousarray(kbf[:, dsl]),
            "vb": np.ascontiguousarray(vbf[:, dsl]),
        })

    if _NC_CACHE is None:
        _NC_CACHE = build_nc()
    nc = _NC_CACHE

    trace = bool(os.environ.get("KERNEL_TRACE"))
    res = run_bass_kernel_spmd(nc, in_maps, core_ids=list(range(NC)),
                               trace=trace)
    if trace and res.exec_time_ns is not None:
        print(f"HW exec time: {res.exec_time_ns} ns")

    out = np.empty((B * S, H), dtype=np.float32)
    for c in range(NC):
        out[c * TOK:(c + 1) * TOK, :] = res.results[c]["outT"].T
    return out.reshape(B, S, H)
